# revision 1
# baseline (speedup 1.0000x reference)
"""ContentGuidedAttention Trainium2 kernel.

Full NxN single-head cross-attention + out-proj + residual + LayerNorm,
for B=4, C=256, H=W=64 (N=4096 tokens), distributed over 8 NeuronCores:
core i handles batch i//2, query-half i%2 (2048 queries, all 4096 keys).
No collectives: K/V are computed redundantly on the two cores sharing a
batch (~5% extra FLOPs).

Layout strategy (all channel-major, zero transposes):
  - Q^T, K^T computed as [C, n] (channels on partitions) in bf16
  - V computed token-major [n, C] in bf16
  - S^T = K Q^T computed as [k, q] psum tiles; exp on ACT -> P^T bf16
  - softmax denominator: contiguous DVE chunk-tree then a ones-vector
    matmul reduces the 128 partitions -> [1, q]
  - reciprocals and rsqrt run on ACT as exp(-ln x) / exp(-0.5 ln x):
    Ln and Exp share one activation-table set, so no table switches
  - row -> all-partition replication via K=1 ones-column matmuls
  - PV: O^T[c, q] = sum_k V[k,c] P^T[k,q]; out-proj keeps channel-major
  - LN entirely per-query-block, overlapped with the next block's
    attention; affine via ACT Identity with per-partition scale/bias
Projection matmuls run in float32r (full PE rate at free dim >= 256).
"""

import numpy as np

import concourse.bass as bass
import concourse.mybir as mybir
import concourse.tile as tile
from concourse import bacc
from concourse.bass import ds, ts
from concourse.bass_utils import run_bass_kernel_spmd

F32 = mybir.dt.float32
F32R = mybir.dt.float32r
BF16 = mybir.dt.bfloat16
AF = mybir.ActivationFunctionType
OP = mybir.AluOpType

B = 4
C = 256
N = 4096          # tokens per batch
NQ = 2048         # queries per core
QB = 512          # query block
NQB = NQ // QB    # 4
NKC = N // 128    # 32 key chunks
NKR = 4           # key ranges (1024 keys each) for K^T / V tiles
SCALE = (C // 8) ** -0.5
LN_EPS = 1e-5

_CACHE = {}


def _build_nc(dbg=False):
    nc = bacc.Bacc("TRN2", target_bir_lowering=False, debug=False)

    low_d = nc.declare_dram_parameter("low", [C, NQ], F32R, isOutput=False)
    high_d = nc.declare_dram_parameter("high", [C, N], F32R, isOutput=False)
    # weights are passed pre-transposed: [c_in, c_out]
    wq_d = nc.declare_dram_parameter("wq", [C, C], F32R, isOutput=False)
    wk_d = nc.declare_dram_parameter("wk", [C, C], F32R, isOutput=False)
    wv_d = nc.declare_dram_parameter("wv", [C, C], F32R, isOutput=False)
    wo_d = nc.declare_dram_parameter("wo", [C, C], F32R, isOutput=False)
    # qb, kb, ob, lng, lnb prepacked host-side as [128, 10]
    pvec_d = nc.declare_dram_parameter("pvec", [128, 10], F32, isOutput=False)
    out_d = nc.declare_dram_parameter("out", [C, NQ], F32, isOutput=True)
    dbg_d = {}
    if dbg:
        for nm, shp, dt_ in [
            ("dbg_rcp", [1, 512], F32), ("dbg_mu", [1, 512], F32),
            ("dbg_var", [1, 512], F32), ("dbg_rstd", [1, 512], F32),
            ("dbg_tT", [128, 512], BF16), ("dbg_ot", [128, 2, QB], F32),
            ("dbg_qt", [128, 2, QB], BF16), ("dbg_kt", [128, 2, 1024], BF16),
            ("dbg_v", [128, 8, C], BF16), ("dbg_pt", [128, 8, QB], BF16),
        ]:
            dbg_d[nm] = nc.declare_dram_parameter(nm, shp, dt_, isOutput=True)

    with tile.TileContext(nc) as tc:
        with (
            tc.tile_pool(name="persist", bufs=1) as pp,
            tc.tile_pool(name="high", bufs=3) as high_pool,
            tc.tile_pool(name="pt", bufs=5) as pt_pool,
            tc.tile_pool(name="ot", bufs=2) as ot_pool,
            tc.tile_pool(name="scratch", bufs=2) as scr_pool,
            tc.tile_pool(name="rowscr", bufs=1) as row_pool,
            tc.tile_pool(name="outsb", bufs=2) as out_pool,
            tc.tile_pool(name="st_ps", bufs=2, space="PSUM") as st_ps,
            tc.tile_pool(name="acc_ps", bufs=3, space="PSUM") as acc_ps,
            tc.tile_pool(name="row_ps", bufs=1, space="PSUM") as row_ps,
        ):
            # ---------------- constants / parameters ----------------
            # one tile per weight matrix ([cin_p, cin_chunk, cout]); K/V
            # weights load first so the K/V projections start ASAP
            pvec = pp.tile([128, 10], F32)
            nc.scalar.dma_start(out=pvec[:, :], in_=pvec_d[:, :])
            wk_sb = pp.tile([128, 2, C], F32R)
            wv_sb = pp.tile([128, 2, C], F32R)
            wq_sb = pp.tile([128, 2, C], F32R)
            wo_sb = pp.tile([128, 2, C], F32R)
            for t, d in [(wk_sb, wk_d), (wv_sb, wv_d), (wq_sb, wq_d),
                         (wo_sb, wo_d)]:
                for j in range(2):
                    nc.scalar.dma_start(out=t[:, j, :], in_=d[ds(j * 128, 128), :])

            # memset cannot emit float32r; stage in f32 and copy (the
            # DVE tensor_copy performs the f32 -> f32r rounding walrus wants)
            stage = pp.tile([128, 128], F32)
            ones1 = pp.tile([1, 128], F32R)      # K=1 replication lhsT
            nc.vector.memset(stage[ds(0, 1), :], 1.0)
            nc.vector.tensor_copy(ones1[:, :], stage[ds(0, 1), :])
            ones128 = pp.tile([128, 1], F32R)    # partition-reduce lhsT (f32r)
            nc.vector.memset(stage[:, 0:1], 1.0)
            nc.vector.tensor_copy(ones128[:, :], stage[:, 0:1])
            ones128b = pp.tile([128, 1], BF16)   # partition-reduce lhsT (bf16)
            nc.vector.memset(ones128b[:, :], 1.0)
            epsb = pp.tile([1, 1], F32)          # LN epsilon bias
            nc.vector.memset(epsb[:, :], LN_EPS)

            QBIAS, KBIAS, OBIAS, LNG, LNB = 0, 2, 4, 6, 8

            # ---------------- K^T / V projections ----------------
            # per 1024-key-range tiles so attention can start early
            kt_sb = [
                pp.tile([128, 2, 1024], BF16, name=f"kt{r}", tag=f"kt{r}")
                for r in range(NKR)
            ]
            v_sb = [
                pp.tile([128, 8, C], BF16, name=f"v{r}", tag=f"v{r}")
                for r in range(NKR)
            ]
            for kr in range(N // 512):
                hi = high_pool.tile([128, 2, 512], F32R)
                for j in range(2):
                    nc.sync.dma_start(
                        out=hi[:, j, :],
                        in_=high_d[ds(j * 128, 128), ds(kr * 512, 512)],
                    )
                r, h = kr // 2, kr % 2
                # K^T: out [cout, k] = sum_cin wk[cin, cout] high[cin, k]
                for c in range(2):
                    kps = st_ps.tile([128, 512], F32, tag="st")
                    for j in range(2):
                        nc.tensor.matmul(
                            out=kps[:, :],
                            lhsT=wk_sb[:, j, ds(c * 128, 128)],
                            rhs=hi[:, j, :],
                            start=(j == 0), stop=(j == 1),
                        )
                    nc.vector.tensor_scalar_add(
                        out=kt_sb[r][:, c, ds(h * 512, 512)],
                        in0=kps[:, :],
                        scalar1=pvec[:, ds(KBIAS + c, 1)],
                    )
                # V: out [k, cout] = sum_cin high[cin, k] wv[cin, cout]
                for u in range(4):
                    vps = st_ps.tile([128, C], F32, tag="st")
                    for j in range(2):
                        nc.tensor.matmul(
                            out=vps[:, :],
                            lhsT=hi[:, j, ds(u * 128, 128)],
                            rhs=wv_sb[:, j, :],
                            start=(j == 0), stop=(j == 1),
                        )
                    nc.scalar.activation(
                        out=v_sb[r][:, h * 4 + u, :], in_=vps[:, :],
                        func=AF.Copy,
                    )

            # ---------------- Q^T projection (all blocks) ----------------
            low_sb = pp.tile([128, 2, NQ], F32R)
            for j in range(2):
                nc.scalar.dma_start(
                    out=low_sb[:, j, :], in_=low_d[ds(j * 128, 128), :]
                )
            qt_all = pp.tile([128, 2, NQ], BF16)
            for qb4 in range(NQB):
                for c in range(2):
                    qps = st_ps.tile([128, QB], F32, tag="st")
                    for j in range(2):
                        nc.tensor.matmul(
                            out=qps[:, :],
                            lhsT=wq_sb[:, j, ds(c * 128, 128)],
                            rhs=low_sb[:, j, ds(qb4 * QB, QB)],
                            start=(j == 0), stop=(j == 1),
                        )
                    nc.vector.tensor_scalar_add(
                        out=qt_all[:, c, ds(qb4 * QB, QB)], in0=qps[:, :],
                        scalar1=pvec[:, ds(QBIAS + c, 1)],
                    )


            # ---------------- main loop over query blocks ----------------
            # Emission is software-pipelined: block b's scalar-chain matmuls
            # (denominator, out-proj, LN stats) are emitted inside block
            # b+1's attention so the in-order PE queue never waits on the
            # DVE/ACT softmax-denominator and LayerNorm chains.

            def attention(b):
                qsl = ds(b * QB, QB)
                quarters = [
                    pt_pool.tile([128, 8, QB], BF16, tag="ptq", name=f"ptq{g}")
                    for g in range(4)
                ]
                for si in range(16):
                    sps = st_ps.tile([128, 2, QB], F32, tag="st")
                    for u in range(2):
                        kc = si * 2 + u
                        for c in range(2):
                            nc.tensor.matmul(
                                out=sps[:, u, :],
                                lhsT=kt_sb[kc // 8][:, c, ds((kc % 8) * 128, 128)],
                                rhs=qt_all[:, c, qsl],
                                start=(c == 0), stop=(c == 1),
                            )
                    nc.scalar.activation(
                        out=quarters[si // 4][:, ds((si % 4) * 2, 2), :],
                        in_=sps[:, :, :],
                        func=AF.Exp,
                        scale=SCALE,
                    )
                return quarters

            def pv(b, quarters):
                ot = ot_pool.tile([128, 2, QB], F32R, tag="ot", name=f"ot{b}")
                for c in range(2):
                    ops = acc_ps.tile([128, QB], F32, tag="acc")
                    for kc in range(NKC):
                        nc.tensor.matmul(
                            out=ops[:, :],
                            lhsT=v_sb[kc // 8][:, kc % 8, ds(c * 128, 128)],
                            rhs=quarters[kc // 8][:, kc % 8, :],
                            start=(kc == 0), stop=(kc == NKC - 1),
                        )
                    nc.vector.tensor_copy(ot[:, c, :], ops[:, :])
                return ot

            def tree(b, quarters):
                # contiguous DVE chunk tree for the softmax denominator.
                # q0..q2 fold while exp of q3 is still streaming; q3 then
                # folds shallowly so the post-attention critical path is
                # only ~2us of DVE work.
                fl = [q[:, :, :].rearrange("p a b -> p (a b)") for q in quarters]
                nc.vector.tensor_add(out=fl[0], in0=fl[0], in1=fl[1])
                nc.vector.tensor_add(out=fl[0], in0=fl[0], in1=fl[2])
                tB = scr_pool.tile([128, 4, QB], BF16, tag="tB")
                nc.vector.tensor_add(
                    out=tB[:, :, :],
                    in0=quarters[0][:, 0:4, :], in1=quarters[0][:, 4:8, :],
                )
                tT2 = scr_pool.tile([128, 2, QB], BF16, tag="tT2")
                nc.vector.tensor_add(
                    out=tT2[:, :, :], in0=tB[:, 0:2, :], in1=tB[:, 2:4, :]
                )
                p012 = scr_pool.tile([128, QB], BF16, tag="p012")
                nc.vector.tensor_add(
                    out=p012[:, :], in0=tT2[:, 0, :], in1=tT2[:, 1, :]
                )
                a3 = scr_pool.tile([128, 4, QB], BF16, tag="tB")
                nc.vector.tensor_add(
                    out=a3[:, :, :],
                    in0=quarters[3][:, 0:4, :], in1=quarters[3][:, 4:8, :],
                )
                b3 = scr_pool.tile([128, 2, QB], BF16, tag="tT2")
                nc.vector.tensor_add(
                    out=b3[:, :, :], in0=a3[:, 0:2, :], in1=a3[:, 2:4, :]
                )
                tT = scr_pool.tile([128, QB], BF16, tag="tT", name=f"tT{b}")
                nc.vector.scalar_tensor_tensor(
                    out=tT[:, :], in0=b3[:, 0, :], scalar=0.0,
                    in1=b3[:, 1, :], op0=OP.add, op1=OP.add,
                )
                nc.vector.tensor_add(
                    out=tT[:, :], in0=tT[:, :], in1=p012[:, :]
                )
                return tT

            def denom_recip(b, tT):
                dn_ps = row_ps.tile([1, QB], F32, tag="row")
                nc.tensor.matmul(
                    out=dn_ps[:, :], lhsT=ones128b[:, :], rhs=tT[:, :],
                    start=True, stop=True,
                )
                # 1/denom = exp(-ln(denom)) on ACT (same table set as Exp)
                lnrow = row_pool.tile([1, QB], F32, tag="lnrow")
                nc.scalar.activation(
                    out=lnrow[:, :], in_=dn_ps[:, :], func=AF.Ln
                )
                rcprow = row_pool.tile([1, QB], F32, tag="rcprow",
                                       name=f"rcprow{b}")
                nc.scalar.activation(
                    out=rcprow[:, :], in_=lnrow[:, :], func=AF.Exp, scale=-1.0
                )
                rcp_rep = scr_pool.tile([128, QB], F32, tag="rcprep",
                                        name=f"rcprep{b}")
                nc.gpsimd.partition_broadcast(rcp_rep[:, :], rcprow[:, :])
                return rcprow, rcp_rep

            def outproj_y(b, ot, rcp_rep):
                qsl = ds(b * QB, QB)
                y_sb = ot_pool.tile([128, 2, QB], F32R, tag="y", name=f"y{b}")
                for c in range(2):
                    pps = acc_ps.tile([128, QB], F32, tag="acc")
                    for j in range(2):
                        nc.tensor.matmul(
                            out=pps[:, :],
                            lhsT=wo_sb[:, j, ds(c * 128, 128)],
                            rhs=ot[:, j, :],
                            start=(j == 0), stop=(j == 1),
                        )
                    ysc = scr_pool.tile([128, QB], F32, tag="scr")
                    nc.vector.tensor_mul(
                        out=ysc[:, :], in0=pps[:, :], in1=rcp_rep[:, :]
                    )
                    nc.vector.scalar_tensor_tensor(
                        out=y_sb[:, c, :],
                        in0=ysc[:, :],
                        scalar=pvec[:, ds(OBIAS + c, 1)],
                        in1=low_sb[:, c, qsl].bitcast(F32),
                        op0=OP.add, op1=OP.add,
                    )
                return y_sb

            def stats_ln(b, y_sb, rcprow):
                qsl = ds(b * QB, QB)
                sy_ps = row_ps.tile([1, QB], F32, tag="row")
                for c in range(2):
                    nc.tensor.matmul(
                        out=sy_ps[:, :],
                        lhsT=ones128[:, :],
                        rhs=y_sb[:, c, :],
                        start=(c == 0), stop=(c == 1),
                    )
                murow = row_pool.tile([1, QB], F32, tag="murow")
                nc.vector.tensor_scalar_mul(
                    out=murow[:, :], in0=sy_ps[:, :], scalar1=1.0 / C
                )
                sy2_ps = row_ps.tile([1, QB], F32, tag="row")
                for c in range(2):
                    ysq = scr_pool.tile([128, QB], F32R, tag="ysq")
                    nc.vector.tensor_mul(
                        out=ysq[:, :],
                        in0=y_sb[:, c, :].bitcast(F32),
                        in1=y_sb[:, c, :].bitcast(F32),
                    )
                    nc.tensor.matmul(
                        out=sy2_ps[:, :],
                        lhsT=ones128[:, :],
                        rhs=ysq[:, :],
                        start=(c == 0), stop=(c == 1),
                    )
                # var = E[y^2] - mu^2 ; rstd = exp(-0.5 ln(var + eps))
                varrow = row_pool.tile([1, QB], F32, tag="varrow")
                nc.vector.tensor_scalar_mul(
                    out=varrow[:, :], in0=sy2_ps[:, :], scalar1=1.0 / C
                )
                mu2row = row_pool.tile([1, QB], F32, tag="mu2row")
                nc.vector.tensor_mul(
                    out=mu2row[:, :], in0=murow[:, :], in1=murow[:, :],
                )
                nc.vector.tensor_sub(
                    out=varrow[:, :], in0=varrow[:, :], in1=mu2row[:, :]
                )
                lnv = row_pool.tile([1, QB], F32, tag="lnv")
                nc.scalar.activation(
                    out=lnv[:, :], in_=varrow[:, :], func=AF.Ln, bias=epsb[:, :]
                )
                rstdrow = row_pool.tile([1, QB], F32, tag="rstdrow")
                nc.scalar.activation(
                    out=rstdrow[:, :], in_=lnv[:, :], func=AF.Exp, scale=-0.5
                )
                if dbg_d and b == NQB - 1:
                    nc.sync.dma_start(out=dbg_d["dbg_rcp"][:, :], in_=rcprow[:, :])
                    nc.sync.dma_start(out=dbg_d["dbg_mu"][:, :], in_=murow[:, :])
                    nc.sync.dma_start(out=dbg_d["dbg_var"][:, :],
                                      in_=varrow[:, :])
                    nc.sync.dma_start(out=dbg_d["dbg_rstd"][:, :],
                                      in_=rstdrow[:, :])
                mu_rep = scr_pool.tile([128, QB], F32, tag="murep")
                nc.gpsimd.partition_broadcast(mu_rep[:, :], murow[:, :])
                rs_rep = scr_pool.tile([128, QB], F32, tag="rsrep")
                nc.gpsimd.partition_broadcast(rs_rep[:, :], rstdrow[:, :])
                for c in range(2):
                    yn = scr_pool.tile([128, QB], F32, tag="scr")
                    nc.vector.tensor_sub(
                        out=yn[:, :],
                        in0=y_sb[:, c, :].bitcast(F32),
                        in1=mu_rep[:, :],
                    )
                    nc.vector.tensor_mul(
                        out=yn[:, :], in0=yn[:, :], in1=rs_rep[:, :]
                    )
                    osb = out_pool.tile([128, QB], F32)
                    nc.vector.tensor_scalar(
                        out=osb[:, :], in0=yn[:, :],
                        scalar1=pvec[:, ds(LNG + c, 1)],
                        scalar2=pvec[:, ds(LNB + c, 1)],
                        op0=OP.mult, op1=OP.add,
                    )
                    nc.scalar.dma_start(
                        out=out_d[ds(c * 128, 128), qsl], in_=osb[:, :]
                    )

            for b in range(NQB):
                quarters = attention(b)
                ot = pv(b, quarters)
                tT = tree(b, quarters)
                rcprow, rcp_rep = denom_recip(b, tT)
                y_b = outproj_y(b, ot, rcp_rep)
                stats_ln(b, y_b, rcprow)
                if dbg_d and b == NQB - 1:
                    nc.sync.dma_start(out=dbg_d["dbg_tT"][:, :], in_=tT[:, :])
                    nc.sync.dma_start(
                        out=dbg_d["dbg_ot"][:, :, :], in_=ot[:, :, :].bitcast(F32)
                    )
                    nc.sync.dma_start(out=dbg_d["dbg_qt"][:, :, :],
                                      in_=qt_all[:, :, 3 * QB:4 * QB])
                    nc.sync.dma_start(
                        out=dbg_d["dbg_kt"][:, :, :], in_=kt_sb[0][:, :, :]
                    )
                    nc.sync.dma_start(
                        out=dbg_d["dbg_v"][:, :, :], in_=v_sb[0][:, :, :]
                    )
                    nc.sync.dma_start(
                        out=dbg_d["dbg_pt"][:, :, :], in_=quarters[3][:, :, :]
                    )

    # Force Exp and Ln to resolve to the one table set containing both
    # (the default chooser alternates exp_and_others <-> natural_log_exp,
    # paying a ~1.3us table load per switch, ~17 loads per kernel).
    import bass_rust as _br
    from concourse.hw_specs import get_activation_tables as _gat

    def _patched_act_loads():
        has_act = any(
            isinstance(i, mybir.InstActivation)
            for blk in nc.main_func.blocks for i in blk.instructions
        )
        if not has_act:
            return
        tables = []
        for name, fns in _gat(nc.m.arch).items():
            if name != "natural_log_exp_and_others":
                fns = fns - {AF.Exp, AF.Ln}
            tables.append((name, fns))
        _br.insert_act_table_loads(nc, tables)

    nc.insert_act_table_loads = _patched_act_loads
    nc.compile()
    return nc


def get_nc(dbg=False):
    key = "nc_dbg" if dbg else "nc"
    if key not in _CACHE:
        _CACHE[key] = _build_nc(dbg)
    return _CACHE[key]


def make_in_maps(low, high, q_w, q_b, k_w, k_b, v_w, v_b, o_w, o_b, ln_g, ln_b):
    low_r = np.asarray(low, np.float32).reshape(B, C, N)
    high_r = np.asarray(high, np.float32).reshape(B, C, N)
    f32 = lambda x: np.ascontiguousarray(np.asarray(x, np.float32))
    # v-bias is exactly equivalent to an out-proj bias shift because the
    # softmax rows sum to one: attn @ (V + 1 vb^T) @ o_w^T = attn @ V @ o_w^T
    # + (o_w @ v_b)^T, so fold it on the host.
    ob_eff = np.asarray(o_b, np.float32) + np.asarray(o_w, np.float32) @ np.asarray(v_b, np.float32)
    pv_cols = []
    for v in [q_b, k_b, ob_eff, ln_g, ln_b]:
        pv_cols.append(np.asarray(v, np.float32).reshape(2, 128).T)
    shared = {
        "wq": f32(np.asarray(q_w, np.float32).T),
        "wk": f32(np.asarray(k_w, np.float32).T),
        "wv": f32(np.asarray(v_w, np.float32).T),
        "wo": f32(np.asarray(o_w, np.float32).T),
        "pvec": f32(np.concatenate(pv_cols, axis=1)),
    }
    in_maps = []
    for i in range(8):
        bidx, h = i // 2, i % 2
        in_maps.append({
            "low": f32(low_r[bidx][:, h * NQ:(h + 1) * NQ]),
            "high": f32(high_r[bidx]),
            **shared,
        })
    return in_maps


def assemble(results):
    out = np.empty((B, C, N), np.float32)
    for i in range(8):
        bidx, h = i // 2, i % 2
        out[bidx][:, h * NQ:(h + 1) * NQ] = results[i]["out"]
    return out.reshape(B, C, 64, 64)


def kernel(**inputs) -> np.ndarray:
    nc = get_nc()
    in_maps = make_in_maps(**inputs)
    res = run_bass_kernel_spmd(nc, in_maps, core_ids=list(range(8)))
    return assemble(res.results)


if __name__ == "__main__":
    pass



# revision 17
# speedup vs baseline: 1.2848x; 1.2848x over previous
"""ContentGuidedAttention Trainium2 kernel (fp8 DoubleRow version).

Full NxN single-head cross-attention + out-proj + residual + LayerNorm,
for B=4, C=256, H=W=64 (N=4096 tokens), distributed over 8 NeuronCores:
core i handles batch i//2, query-half i%2 (2048 queries, all 4096 keys).
No collectives: K/V are computed redundantly on the two cores sharing a
batch.

v2 design (fp8):
  - ALL matmuls run fp8e4 with MatmulPerfMode.DoubleRow: each op
    contracts 2x128 partitions at 0.5 cycles/row.  Layouts [128, 2, *]
    put the two contraction tiles on dim1 as DoubleRow requires.
  - k_b is dropped: (q+qb).(k+kb) differs from (q+qb).k by per-query
    logit constants, which cancel exactly in softmax.  v_b is folded
    into o_b host-side (softmax rows sum to 1).
  - softmax denominator: an all-ones [128,2,128] fp8 lhsT turns the
    DoubleRow reduction into a partition-REPLICATED [128,512] psum row,
    so 1/denom comes from one DVE reciprocal with no partition
    broadcast at all.  LN stats (sum_y, sum_y2) use the same trick with
    an all-ones [128,128] f32r lhsT, so mu/var/rstd are computed
    column-replicated at identical DVE/ACT cost (engines process all
    128 partitions in parallel).  GpSimd has no work left in the steady
    state, so it takes the yn normalize chain (SBUF-only ops).
  - exp runs on ACT (psum [128,2,512] -> fp8 quarters); ACT is the
    bottleneck (~70us), so everything else is kept off it.
  - K/V/Q psum->SBUF fp8 copies on DVE (GPSIMD cannot access PSUM).
  - PSUM banks: st 2x[128,2,512] (4) + acc pvps (2) + dn (1) +
    stats (1) = 8.
  - emission is software-pipelined: att(0) interleaves into the K/V
    projection loop (same st-tag rotation), PV(b) interleaves into
    att(b) at si>=4 (quarter g complete after si=4g+3), out-proj(b)
    lands in the st rotation inside att(b+1), and the whole epilogue
    of block b is emitted across att(b+1)'s si hooks so the in-order
    PE/ACT queues never wait on the scalar chains.
"""

import numpy as np
import ml_dtypes

import concourse.bass as bass
import concourse.mybir as mybir
import concourse.tile as tile
from concourse import bacc
from concourse.bass import ds, ts
from concourse.bass_utils import run_bass_kernel_spmd

F32 = mybir.dt.float32
F32R = mybir.dt.float32r
FP8 = mybir.dt.float8e4
AF = mybir.ActivationFunctionType
OP = mybir.AluOpType
DR = mybir.MatmulPerfMode.DoubleRow

B = 4
C = 256
N = 4096          # tokens per batch
NQ = 2048         # queries per core
QB = 512          # query block
NQB = NQ // QB    # 4
SCALE = (C // 8) ** -0.5
LN_EPS = 1e-5

_CACHE = {}


def _build_nc(dbg=False):
    nc = bacc.Bacc("TRN2", target_bir_lowering=False, debug=False)

    low8_d = nc.declare_dram_parameter("low8", [C, NQ], FP8, isOutput=False)
    lowf_d = nc.declare_dram_parameter("lowf", [C, NQ], F32R, isOutput=False)
    high8_d = nc.declare_dram_parameter("high8", [C, N], FP8, isOutput=False)
    # weights pre-transposed host-side: [c_in, c_out], fp8
    wq_d = nc.declare_dram_parameter("wq8", [C, C], FP8, isOutput=False)
    wk_d = nc.declare_dram_parameter("wk8", [C, C], FP8, isOutput=False)
    wv_d = nc.declare_dram_parameter("wv8", [C, C], FP8, isOutput=False)
    wo_d = nc.declare_dram_parameter("wo8", [C, C], FP8, isOutput=False)
    # qb, ob_eff, ln_g, ln_b prepacked host-side as [128, 8]
    pvec_d = nc.declare_dram_parameter("pvec", [128, 8], F32, isOutput=False)
    out_d = nc.declare_dram_parameter("out", [C, NQ], F32, isOutput=True)
    dbg_d = {}
    if dbg:
        for nm, shp, dt_ in [
            ("dbg_rcp", [1, 512], F32), ("dbg_mu", [1, 512], F32),
            ("dbg_var", [1, 512], F32), ("dbg_ot", [128, 2, QB], F32),
            ("dbg_qt", [128, 2, QB], FP8), ("dbg_kt", [128, 2, 1024], FP8),
            ("dbg_v", [128, 8, C], FP8), ("dbg_pt", [128, 8, QB], FP8),
        ]:
            dbg_d[nm] = nc.declare_dram_parameter(nm, shp, dt_, isOutput=True)

    QBIAS, OBIAS, LNG, LNB = 0, 2, 4, 6

    with tile.TileContext(nc) as tc:
        with (
            tc.tile_pool(name="persist", bufs=1) as pp,
            tc.tile_pool(name="high", bufs=3) as high_pool,
            tc.tile_pool(name="pt", bufs=5) as pt_pool,
            tc.tile_pool(name="ot", bufs=2) as ot_pool,
            tc.tile_pool(name="scratch", bufs=2) as scr_pool,
            tc.tile_pool(name="outsb", bufs=2) as out_pool,
            tc.tile_pool(name="st_ps", bufs=2, space="PSUM") as st_ps,    # 4 banks
            tc.tile_pool(name="acc_ps", bufs=1, space="PSUM") as acc_ps,  # 2 banks
            tc.tile_pool(name="dn_ps", bufs=1, space="PSUM") as dn_ps,    # 1 bank
            tc.tile_pool(name="sy_ps", bufs=1, space="PSUM") as sy_ps,    # 1 bank
        ):
            # ---------------- constants / parameters ----------------
            pvec = pp.tile([128, 8], F32)
            nc.sync.dma_start(out=pvec[:, :], in_=pvec_d[:, :])
            wk_sb = pp.tile([128, 2, C], FP8)
            wv_sb = pp.tile([128, 2, C], FP8)
            wq_sb = pp.tile([128, 2, C], FP8)
            wo_sb = pp.tile([128, 2, C], FP8)
            for t, d in [(wq_sb, wq_d), (wk_sb, wk_d), (wv_sb, wv_d)]:
                for j in range(2):
                    nc.sync.dma_start(out=t[:, j, :], in_=d[ds(j * 128, 128), :])
            low8_sb = pp.tile([128, 2, NQ], FP8)
            for j in range(2):
                nc.sync.dma_start(
                    out=low8_sb[:, j, :], in_=low8_d[ds(j * 128, 128), :]
                )

            # all-ones lhsTs: fp8 [128,2,128] for the partition-replicated
            # denominator reduction, f32r [128,128] for LN stats.
            stage = pp.tile([128, 256], F32)
            nc.vector.memset(stage[:, :], 1.0)
            ones8 = pp.tile([128, 2, 128], FP8)
            nc.vector.tensor_copy(ones8[:, :, :], stage[:, :])
            ones128 = pp.tile([128, 128], F32R)
            nc.vector.tensor_copy(ones128[:, :], stage[:, 0:128])
            epsb = pp.tile([128, 1], F32)
            nc.vector.memset(epsb[:, :], LN_EPS)

            kt_sb = [
                pp.tile([128, 2, 1024], FP8, name=f"kt{r}", tag=f"kt{r}")
                for r in range(4)
            ]
            v_sb = [
                pp.tile([128, 8, C], FP8, name=f"v{r}", tag=f"v{r}")
                for r in range(4)
            ]
            low_sb = pp.tile([128, 2, NQ], F32R)
            qt_all = pp.tile([128, 2, NQ], FP8)

            state = {}

            # ---------------- emission helpers ----------------
            def qproj_emit():
                for qb4 in range(NQB):
                    qps = st_ps.tile([128, 2, QB], F32, tag="st")
                    for c in range(2):
                        nc.tensor.matmul(
                            out=qps[:, c, :],
                            lhsT=wq_sb[:, :, ds(c * 128, 128)],
                            rhs=low8_sb[:, :, ds(qb4 * QB, QB)],
                            start=True, stop=True, perf_mode=DR,
                        )
                    for c in range(2):
                        nc.vector.tensor_scalar_add(
                            out=qt_all[:, c, ds(qb4 * QB, QB)],
                            in0=qps[:, c, :],
                            scalar1=pvec[:, ds(QBIAS + c, 1)],
                        )

            def kv_emit(kr):
                hi = high_pool.tile([128, 2, 512], FP8)
                for j in range(2):
                    nc.sync.dma_start(
                        out=hi[:, j, :],
                        in_=high8_d[ds(j * 128, 128), ds(kr * 512, 512)],
                    )
                r, h = kr // 2, kr % 2
                # K^T [cout, k]: one DoubleRow matmul per c-half
                kps = st_ps.tile([128, 2, 512], F32, tag="st")
                for c in range(2):
                    nc.tensor.matmul(
                        out=kps[:, c, :],
                        lhsT=wk_sb[:, :, ds(c * 128, 128)],
                        rhs=hi[:, :, :],
                        start=True, stop=True, perf_mode=DR,
                    )
                nc.vector.tensor_copy(
                    kt_sb[r][:, :, ds(h * 512, 512)], kps[:, :, :]
                )
                # V [k, cout]: one DoubleRow per 128-token chunk, packed
                # 2 chunks per st tile
                for g in range(2):
                    vps = st_ps.tile([128, 2, QB], F32, tag="st")
                    for u in range(2):
                        nc.tensor.matmul(
                            out=vps[:, u, 0:C],
                            lhsT=hi[:, :, ds((g * 2 + u) * 128, 128)],
                            rhs=wv_sb[:, :, :],
                            start=True, stop=True, perf_mode=DR,
                        )
                    nc.vector.tensor_copy(
                        v_sb[r][:, ds(h * 4 + g * 2, 2), :],
                        vps[:, :, 0:C],
                    )

            def att_begin(b):
                quarters = [
                    pt_pool.tile([128, 8, QB], FP8, tag="ptq",
                                 name=f"ptq{b}_{g}")
                    for g in range(4)
                ]
                dnps = dn_ps.tile([128, QB], F32, tag="dn", name=f"dn{b}")
                pvps = acc_ps.tile([128, 2, QB], F32, tag="acc",
                                   name=f"pvps{b}")
                state[b] = (quarters, dnps)
                state[(b, "pvps")] = pvps

            def att_si(b, si, hooks):
                quarters, dnps = state[b]
                qsl = ds(b * QB, QB)
                sps = st_ps.tile([128, 2, QB], F32, tag="st")
                for u in range(2):
                    kc = si * 2 + u
                    nc.tensor.matmul(
                        out=sps[:, u, :],
                        lhsT=kt_sb[kc // 8][:, :, ds((kc % 8) * 128, 128)],
                        rhs=qt_all[:, :, qsl],
                        start=True, stop=True, perf_mode=DR,
                    )
                nc.scalar.activation(
                    out=quarters[si // 4][:, ds((si % 4) * 2, 2), :],
                    in_=sps[:, :, :],
                    func=AF.Exp,
                    scale=SCALE,
                )
                # denominator, replicated across all 128 partitions
                nc.tensor.matmul(
                    out=dnps[:, :],
                    lhsT=ones8[:, :, :],
                    rhs=quarters[si // 4][:, ds((si % 4) * 2, 2), :],
                    start=(si == 0), stop=(si == 15),
                    perf_mode=DR,
                )
                for th in hooks.get(si, []):
                    th()

            def pv_ops(b, c, g):
                quarters, _ = state[b]
                ops = state[(b, "pvps")]
                for uu in range(4):
                    nc.tensor.matmul(
                        out=ops[:, c, :],
                        lhsT=v_sb[g][:, ds(2 * uu, 2), ds(c * 128, 128)],
                        rhs=quarters[g][:, ds(2 * uu, 2), :],
                        start=(g == 0 and uu == 0), stop=(g == 3 and uu == 3),
                        perf_mode=DR,
                    )

            def rcp_emit(b):
                # 1/denom: IEEE-exact DVE reciprocal, already replicated
                _, dnps = state[b]
                rcp_rep = scr_pool.tile([128, QB], F32, tag="rcprep",
                                        name=f"rcprep{b}")
                nc.vector.reciprocal(rcp_rep[:, :], dnps[:, :])
                state[(b, "rcp")] = rcp_rep

            def otcopy_emit(b):
                ops = state[(b, "pvps")]
                ot = ot_pool.tile([128, 2, QB], FP8, tag="ot", name=f"ot{b}")
                nc.vector.tensor_copy(ot[:, :, :], ops[:, :, :])
                state[(b, "ot")] = ot

            def finish_block(b):
                pv_ops(b, 0, 3)
                pv_ops(b, 1, 3)
                rcp_emit(b)
                otcopy_emit(b)

            def outproj_emit(b):
                # pps lives in the st rotation: its slot is recycled two
                # sps-calls later, whose matmuls then wait on y(b)'s two
                # psum reads -- which complete well before that si.
                ot = state[(b, "ot")]
                pps = st_ps.tile([128, 2, QB], F32, tag="st",
                                 name=f"pps{b}")
                for c in range(2):
                    nc.tensor.matmul(
                        out=pps[:, c, :],
                        lhsT=wo_sb[:, :, ds(c * 128, 128)],
                        rhs=ot[:, :, :],
                        start=True, stop=True, perf_mode=DR,
                    )
                state[(b, "pps")] = pps

            def y_emit(b):
                qsl = ds(b * QB, QB)
                pps = state[(b, "pps")]
                rcp_rep = state[(b, "rcp")]
                y_sb = ot_pool.tile([128, 2, QB], F32R, tag="y", name=f"y{b}")
                for c in range(2):
                    ysc = scr_pool.tile([128, QB], F32, tag="scr")
                    nc.vector.tensor_mul(
                        out=ysc[:, :], in0=pps[:, c, :], in1=rcp_rep[:, :]
                    )
                    nc.vector.scalar_tensor_tensor(
                        out=y_sb[:, c, :],
                        in0=ysc[:, :],
                        scalar=pvec[:, ds(OBIAS + c, 1)],
                        in1=low_sb[:, c, qsl].bitcast(F32),
                        op0=OP.add, op1=OP.add,
                    )
                state[(b, "y")] = y_sb

            def sy_emit(b):
                y_sb = state[(b, "y")]
                syps = sy_ps.tile([128, QB], F32, tag="sy", name=f"sy{b}")
                for c in range(2):
                    nc.tensor.matmul(
                        out=syps[:, :],
                        lhsT=ones128[:, :],
                        rhs=y_sb[:, c, :],
                        start=(c == 0), stop=(c == 1),
                    )
                state[(b, "syps")] = syps

            def mu_emit(b):
                syps = state[(b, "syps")]
                mu_rep = scr_pool.tile([128, QB], F32, tag="murep")
                nc.vector.tensor_scalar_mul(
                    out=mu_rep[:, :], in0=syps[:, :], scalar1=1.0 / C
                )
                state[(b, "mu")] = mu_rep

            def sy2_emit(b):
                # y^2 on GpSimd (SBUF-only op; DVE is the busier engine)
                y_sb = state[(b, "y")]
                sy2ps = sy_ps.tile([128, QB], F32, tag="sy", name=f"sy2{b}")
                for c in range(2):
                    ysq = scr_pool.tile([128, QB], F32R, tag="ysq")
                    nc.gpsimd.tensor_mul(
                        out=ysq[:, :],
                        in0=y_sb[:, c, :].bitcast(F32),
                        in1=y_sb[:, c, :].bitcast(F32),
                    )
                    nc.tensor.matmul(
                        out=sy2ps[:, :],
                        lhsT=ones128[:, :],
                        rhs=ysq[:, :],
                        start=(c == 0), stop=(c == 1),
                    )
                state[(b, "sy2ps")] = sy2ps

            def var_emit(b):
                sy2ps = state[(b, "sy2ps")]
                mu_rep = state[(b, "mu")]
                mu2 = scr_pool.tile([128, QB], F32, tag="mu2")
                nc.vector.tensor_mul(
                    out=mu2[:, :], in0=mu_rep[:, :], in1=mu_rep[:, :],
                )
                var_rep = scr_pool.tile([128, QB], F32, tag="varrep",
                                        name=f"var{b}")
                nc.vector.scalar_tensor_tensor(
                    out=var_rep[:, :], in0=sy2ps[:, :],
                    scalar=1.0 / C, in1=mu2[:, :],
                    op0=OP.mult, op1=OP.subtract,
                )
                state[(b, "var")] = var_rep

            def rstd_emit(b):
                var_rep = state[(b, "var")]
                lnv = scr_pool.tile([128, QB], F32, tag="lnv")
                nc.scalar.activation(
                    out=lnv[:, :], in_=var_rep[:, :], func=AF.Ln,
                    bias=epsb[:, :]
                )
                rs_rep = scr_pool.tile([128, QB], F32, tag="rsrep")
                nc.scalar.activation(
                    out=rs_rep[:, :], in_=lnv[:, :], func=AF.Exp, scale=-0.5
                )
                state[(b, "rs")] = rs_rep

            def yn_emit(b, engs):
                qsl = ds(b * QB, QB)
                y_sb = state[(b, "y")]
                mu_rep = state[(b, "mu")]
                rs_rep = state[(b, "rs")]
                for c in range(2):
                    eng = engs[c]
                    yn = scr_pool.tile([128, QB], F32, tag=f"ynscr{c}")
                    eng.tensor_sub(
                        out=yn[:, :],
                        in0=y_sb[:, c, :].bitcast(F32),
                        in1=mu_rep[:, :],
                    )
                    eng.tensor_mul(
                        out=yn[:, :], in0=yn[:, :], in1=rs_rep[:, :]
                    )
                    osb = out_pool.tile([128, QB], F32)
                    eng.tensor_scalar(
                        out=osb[:, :], in0=yn[:, :],
                        scalar1=pvec[:, ds(LNG + c, 1)],
                        scalar2=pvec[:, ds(LNB + c, 1)],
                        op0=OP.mult, op1=OP.add,
                    )
                    nc.sync.dma_start(
                        out=out_d[ds(c * 128, 128), qsl], in_=osb[:, :]
                    )

            def pv_hooks(b):
                return {
                    4: [lambda: pv_ops(b, 0, 0), lambda: pv_ops(b, 1, 0)],
                    8: [lambda: pv_ops(b, 0, 1), lambda: pv_ops(b, 1, 1)],
                    12: [lambda: pv_ops(b, 0, 2), lambda: pv_ops(b, 1, 2)],
                }

            # ---------------- emission ----------------
            # Q-proj first (att(0) needs qt), then K/V interleaved with
            # att(0): after kr pair p, key chunks 0..512(p+1) exist, so
            # si = 4p..4p+3 of att(0) can be emitted.
            qproj_emit()
            att_begin(0)
            hooks0 = pv_hooks(0)
            for kr in range(8):
                kv_emit(kr)
                if kr % 2 == 1:
                    for si in range(2 * (kr - 1), 2 * (kr - 1) + 4):
                        att_si(0, si, hooks0)
            # wo8 (needed ~20us in) and lowf residual (needed by y(0))
            for j in range(2):
                nc.sync.dma_start(out=wo_sb[:, j, :],
                                  in_=wo_d[ds(j * 128, 128), :])
            for j in range(2):
                for hh in range(2):
                    nc.sync.dma_start(
                        out=low_sb[:, j, ds(hh * 1024, 1024)],
                        in_=lowf_d[ds(j * 128, 128), ds(hh * 1024, 1024)],
                    )
            finish_block(0)

            for b in range(1, NQB):
                att_begin(b)
                hooks = pv_hooks(b)
                hooks[2] = [lambda bb=b - 1: outproj_emit(bb)]
                hooks[3] = [lambda bb=b - 1: y_emit(bb)]
                hooks[4].insert(0, lambda bb=b - 1: sy_emit(bb))
                hooks[5] = [lambda bb=b - 1: mu_emit(bb)]
                hooks[6] = [lambda bb=b - 1: sy2_emit(bb)]
                hooks[7] = [lambda bb=b - 1: var_emit(bb)]
                hooks[8].insert(0, lambda bb=b - 1: rstd_emit(bb))
                hooks[10] = [
                    lambda bb=b - 1: yn_emit(bb, (nc.gpsimd, nc.gpsimd))
                ]
                for si in range(16):
                    att_si(b, si, hooks)
                finish_block(b)

            bl = NQB - 1
            outproj_emit(bl)
            y_emit(bl)
            sy_emit(bl)
            mu_emit(bl)
            sy2_emit(bl)
            var_emit(bl)
            rstd_emit(bl)
            yn_emit(bl, (nc.vector, nc.gpsimd))

            if dbg_d:
                quarters, _ = state[bl]
                nc.sync.dma_start(out=dbg_d["dbg_rcp"][:, :],
                                  in_=state[(bl, "rcp")][ds(0, 1), :])
                nc.sync.dma_start(out=dbg_d["dbg_mu"][:, :],
                                  in_=state[(bl, "mu")][ds(0, 1), :])
                nc.sync.dma_start(out=dbg_d["dbg_var"][:, :],
                                  in_=state[(bl, "var")][ds(0, 1), :])
                nc.sync.dma_start(
                    out=dbg_d["dbg_ot"][:, :, :],
                    in_=state[(bl, "pps")][:, :, :],
                )
                nc.sync.dma_start(out=dbg_d["dbg_qt"][:, :, :],
                                  in_=qt_all[:, :, 3 * QB:4 * QB])
                nc.sync.dma_start(out=dbg_d["dbg_kt"][:, :, :],
                                  in_=kt_sb[0][:, :, :])
                nc.sync.dma_start(out=dbg_d["dbg_v"][:, :, :],
                                  in_=v_sb[0][:, :, :])
                nc.sync.dma_start(out=dbg_d["dbg_pt"][:, :, :],
                                  in_=quarters[3][:, :, :])

    # Force Exp and Ln to resolve to the one table set containing both
    # (the default chooser alternates exp_and_others <-> natural_log_exp,
    # paying a ~1.3us table load per switch).
    import bass_rust as _br
    from concourse.hw_specs import get_activation_tables as _gat

    def _patched_act_loads():
        has_act = any(
            isinstance(i, mybir.InstActivation)
            for blk in nc.main_func.blocks for i in blk.instructions
        )
        if not has_act:
            return
        tables = []
        for name, fns in _gat(nc.m.arch).items():
            if name != "natural_log_exp_and_others":
                fns = fns - {AF.Exp, AF.Ln}
            tables.append((name, fns))
        _br.insert_act_table_loads(nc, tables)

    nc.insert_act_table_loads = _patched_act_loads
    nc.compile()
    return nc


def get_nc(dbg=False):
    key = "nc_dbg" if dbg else "nc"
    if key not in _CACHE:
        _CACHE[key] = _build_nc(dbg)
    return _CACHE[key]


def make_in_maps(low, high, q_w, q_b, k_w, k_b, v_w, v_b, o_w, o_b, ln_g, ln_b):
    low_r = np.asarray(low, np.float32).reshape(B, C, N)
    high_r = np.asarray(high, np.float32).reshape(B, C, N)
    f32 = lambda x: np.ascontiguousarray(np.asarray(x, np.float32))
    fp8 = lambda x: np.ascontiguousarray(
        np.asarray(x, np.float32)).astype(ml_dtypes.float8_e4m3)
    # v-bias folds into the out-proj bias (softmax rows sum to one);
    # k-bias cancels in softmax entirely (per-query logit constants).
    ob_eff = (np.asarray(o_b, np.float32)
              + np.asarray(o_w, np.float32) @ np.asarray(v_b, np.float32))
    pv_cols = []
    for v in [q_b, ob_eff, ln_g, ln_b]:
        pv_cols.append(np.asarray(v, np.float32).reshape(2, 128).T)
    shared = {
        "wq8": fp8(np.asarray(q_w, np.float32).T),
        "wk8": fp8(np.asarray(k_w, np.float32).T),
        "wv8": fp8(np.asarray(v_w, np.float32).T),
        "wo8": fp8(np.asarray(o_w, np.float32).T),
        "pvec": f32(np.concatenate(pv_cols, axis=1)),
    }
    in_maps = []
    for i in range(8):
        bidx, h = i // 2, i % 2
        in_maps.append({
            "low8": fp8(low_r[bidx][:, h * NQ:(h + 1) * NQ]),
            "lowf": f32(low_r[bidx][:, h * NQ:(h + 1) * NQ]),
            "high8": fp8(high_r[bidx]),
            **shared,
        })
    return in_maps


def assemble(results):
    out = np.empty((B, C, N), np.float32)
    for i in range(8):
        bidx, h = i // 2, i % 2
        out[bidx][:, h * NQ:(h + 1) * NQ] = results[i]["out"]
    return out.reshape(B, C, 64, 64)


def kernel(**inputs) -> np.ndarray:
    nc = get_nc()
    in_maps = make_in_maps(**inputs)
    res = run_bass_kernel_spmd(nc, in_maps, core_ids=list(range(8)))
    return assemble(res.results)


if __name__ == "__main__":
    pass
